# revision 1
# baseline (speedup 1.0000x reference)
"""Block-diagonal complex matmul kernel for trn2 (8 NeuronCores).

Reference computation:
  xp = take(x, perm_idx, axis=-2).reshape(B, 2, M, S)
  y_re = xp_re @ hr1 + xp_im @ hi1   (per block a of M)
  y_im = xp_re @ hi2 + xp_im @ hr2
  out  = stack([y_re, y_im], 1).reshape(B, 2, N, R)

Sharding: block dim M=1024 split across 8 cores (128 blocks each).
Permutation gather + all layout shuffles happen host-side in numpy.

Per-core device kernel, per block a:
  psum[16, 256] = xT_re[:, a] .T @ [hr1[a] | hi2[a]]   (start)
                + xT_im[:, a] .T @ [hi1[a] | hr2[a]]   (stop)
  -> cols 0:128 = y_re[a], cols 128:256 = y_im[a]
"""

import os
import numpy as np

B = 16
N = 4096
R = 32
M = 1024   # blocks
S = 128    # block size (contract dim)
NCORES = 8
MLOC = M // NCORES   # 128 blocks per core
NB = 4               # blocks per weight DMA group (1 MiB per dma_start)
NGRP = MLOC // NB

_NC_CACHE = {}


def _build_nc():
    import concourse.bacc as bacc
    import concourse.bass as bass
    import concourse.mybir as mybir
    from concourse import tile

    mm_dt = mybir.dt.float16
    nc = bacc.Bacc(None, target_bir_lowering=False)

    # x operands: per block 48 cols = [x_hi(16) | pad(16) | x_lo(16)] fp16 so
    # the x_lo product rows land at PSUM partition 32 (partition base must be
    # a multiple of 32 for the later DVE read).
    XC = 3 * B
    xrp = nc.dram_tensor("xrp", [S, MLOC * XC], mm_dt, kind="ExternalInput")
    xip = nc.dram_tensor("xip", [S, MLOC * XC], mm_dt, kind="ExternalInput")
    # weights: per block 1024 fp16 cols = [W1_hi | W2_hi | W1_lo | W2_lo]
    # with W1 = [hr1 | hi2], W2 = [hi1 | hr2]
    WC = 8 * S  # 1024 cols per block
    w = nc.dram_tensor("w", [S, MLOC * WC], mm_dt, kind="ExternalInput")
    y = nc.dram_tensor("y", [B, MLOC * 2 * S], mybir.dt.float32, kind="ExternalOutput")

    with tile.TileContext(nc) as tc:
        with (
            tc.tile_pool(name="xp", bufs=1) as xpool,
            tc.tile_pool(name="wp", bufs=6) as wpool,
            tc.tile_pool(name="op", bufs=4) as opool,
            tc.tile_pool(name="ps", bufs=8, space=bass.MemorySpace.PSUM) as ps,
        ):
            xrp_t = xpool.tile([S, MLOC * XC], mm_dt, name="xrp_t")
            xip_t = xpool.tile([S, MLOC * XC], mm_dt, name="xip_t")
            nc.sync.dma_start(xrp_t[:], xrp[:])
            nc.sync.dma_start(xip_t[:], xip[:])
            for g in range(NGRP):
                wt = wpool.tile([S, NB * WC], mm_dt)
                nc.sync.dma_start(wt[:], w[:, g * NB * WC:(g + 1) * NB * WC])
                ot = opool.tile([B, NB * 2 * S], mybir.dt.float32)
                for i in range(NB):
                    a = g * NB + i
                    c0 = i * WC
                    w1h = wt[:, c0:c0 + 2 * S]
                    w2h = wt[:, c0 + 2 * S:c0 + 4 * S]
                    w1l = wt[:, c0 + 4 * S:c0 + 6 * S]
                    w2l = wt[:, c0 + 6 * S:c0 + 8 * S]
                    xs = slice(a * XC, (a + 1) * XC)     # [hi|pad|lo] 48 cols
                    xh = slice(a * XC, a * XC + B)       # hi 16 cols
                    # psum rows 0:16 accumulate y_hi terms; rows 32:48 the
                    # x_lo correction. Final y = rows[0:16] + rows[32:48].
                    pt = ps.tile([3 * B, 2 * S], mybir.dt.float32)
                    nc.tensor.matmul(pt[:], xrp_t[:, xs], w1h, start=True, stop=False)
                    nc.tensor.matmul(pt[:], xip_t[:, xs], w2h, start=False, stop=False)
                    nc.tensor.matmul(pt[:B], xrp_t[:, xh], w1l, start=False, stop=False)
                    nc.tensor.matmul(pt[:B], xip_t[:, xh], w2l, start=False, stop=True)
                    # DVE may read only one PSUM operand: stage lo-rows via ACT
                    lo = opool.tile([B, 2 * S], mybir.dt.float32, name="lo", tag="lo")
                    nc.scalar.copy(lo[:], pt[2 * B:])
                    nc.vector.tensor_add(
                        ot[:, i * 2 * S:(i + 1) * 2 * S], pt[:B], lo[:]
                    )
                nc.sync.dma_start(y[:, g * NB * 2 * S:(g + 1) * NB * 2 * S], ot[:])
    nc.compile()
    return nc


def kernel(x, hr1, hi1, hr2, hi2, perm_idx):
    from concourse.bass_utils import run_bass_kernel_spmd

    if "nc" not in _NC_CACHE:
        _NC_CACHE["nc"] = _build_nc()
    nc = _NC_CACHE["nc"]

    x = np.asarray(x, dtype=np.float32)
    perm_idx = np.asarray(perm_idx)
    # host-side permutation gather + regroup into M blocks of size S
    xp = x[:, :, perm_idx, :].reshape(B, 2, M, S)

    def split16(v):
        hi = v.astype(np.float16)
        lo = (v - hi.astype(np.float32)).astype(np.float16)
        return hi, lo

    in_maps = []
    for c in range(NCORES):
        a0 = c * MLOC
        sl = slice(a0, a0 + MLOC)
        # [B, MLOC, S] -> [S(j), MLOC, B] -> [S, MLOC*B]
        xre = np.ascontiguousarray(
            np.transpose(xp[:, 0, sl, :], (2, 1, 0))
        ).reshape(S, MLOC * B)
        xim = np.ascontiguousarray(
            np.transpose(xp[:, 1, sl, :], (2, 1, 0))
        ).reshape(S, MLOC * B)
        xrh, xrl = split16(xre)
        xih, xil = split16(xim)
        # per block 48 stationary cols: [x_hi(16) | pad(16) | x_lo(16)]
        zpad = np.zeros((S, MLOC, B), dtype=np.float16)
        xrpk = np.concatenate(
            [xrh.reshape(S, MLOC, B), zpad, xrl.reshape(S, MLOC, B)], axis=2
        ).reshape(S, MLOC * 3 * B)
        xipk = np.concatenate(
            [xih.reshape(S, MLOC, B), zpad, xil.reshape(S, MLOC, B)], axis=2
        ).reshape(S, MLOC * 3 * B)
        # W1 = [hr1 | hi2], W2 = [hi1 | hr2]; per block [W1h | W2h | W1l | W2l]
        w1 = np.concatenate([hr1[sl], hi2[sl]], axis=2)
        w2 = np.concatenate([hi1[sl], hr2[sl]], axis=2)
        w1h, w1l = split16(w1)
        w2h, w2l = split16(w2)
        wc = np.concatenate([w1h, w2h, w1l, w2l], axis=2)  # [MLOC, S, 8S]
        wc = np.ascontiguousarray(np.transpose(wc, (1, 0, 2))).reshape(S, MLOC * 8 * S)
        in_maps.append({"xrp": np.ascontiguousarray(xrpk),
                        "xip": np.ascontiguousarray(xipk), "w": wc})

    trace = bool(os.environ.get("KERNEL_TRACE"))
    kwargs = {}
    if trace:
        kwargs["tmpdir"] = os.environ.get("KERNEL_TRACE_DIR") or None
    res = run_bass_kernel_spmd(nc, in_maps, core_ids=list(range(NCORES)), trace=trace, **kwargs)
    if trace and res.exec_time_ns is not None:
        print(f"HW exec time: {res.exec_time_ns} ns")
        _NC_CACHE["exec_time_ns"] = res.exec_time_ns
        _NC_CACHE["profile"] = res

    out = np.empty((B, 2, M, S), dtype=np.float32)
    for c in range(NCORES):
        a0 = c * MLOC
        yc = res.results[c]["y"].reshape(B, MLOC, 2, S)
        out[:, 0, a0:a0 + MLOC, :] = yc[:, :, 0, :]
        out[:, 1, a0:a0 + MLOC, :] = yc[:, :, 1, :]
    return out.reshape(B, 2, N, R)



# revision 3
# speedup vs baseline: 2.4967x; 2.4967x over previous
"""Block-diagonal complex matmul kernel for trn2 (8 NeuronCores).

Reference computation:
  xp = take(x, perm_idx, axis=-2).reshape(B, 2, M, S)
  y_re = xp_re @ hr1 + xp_im @ hi1   (per block a of M)
  y_im = xp_re @ hi2 + xp_im @ hr2
  out  = stack([y_re, y_im], 1).reshape(B, 2, N, R)

Sharding: block dim M=1024 split across 8 cores (128 blocks each).
Permutation gather + all layout shuffles happen host-side in numpy.

Everything on-device is fp16 (correctness gate is 2e-2; fp16 gives ~1e-3).

Per-core device kernel, per block a:
  psum[16, 256] = x_re[:, a].T @ [hr1[a] | hi2[a]]   (start)
                + x_im[:, a].T @ [hi1[a] | hr2[a]]   (stop)
  -> cols 0:128 = y_re[a], cols 128:256 = y_im[a]

PSUM packing: 8 blocks per [128, 512] bank — block i at partition group
32*(i%4) (tensor-engine col tiling) and col half 256*(i//4).  One
128-partition DVE copy (fp32->fp16) per bank into an SBUF staging tile;
512KB output DMAs on the ACT ring (weights stream on the SP ring).
"""

import os
import numpy as np

B = 16
N = 4096
R = 32
M = 1024   # blocks
S = 128    # block size (contract dim)
NCORES = 8
MLOC = M // NCORES   # 128 blocks per core
NB = 16              # blocks per weight DMA group (2 MiB fp16)
NGRP = MLOC // NB    # 8 weight groups
BPB = 8              # blocks per PSUM bank
NBANK = MLOC // BPB  # 16 banks
BANKS_PER_STORE = 4  # y store granularity: [128, 2048] fp16 = 512 KiB

_NC_CACHE = {}


def _build_nc():
    import concourse.bacc as bacc
    import concourse.bass as bass
    import concourse.mybir as mybir
    from concourse import tile

    f16 = mybir.dt.float16
    f32 = mybir.dt.float32
    nc = bacc.Bacc(None, target_bir_lowering=False)

    # stationary x: col a*16+b holds x[b, block a, j=partition]
    xr = nc.dram_tensor("xr", [S, MLOC * B], f16, kind="ExternalInput")
    xi = nc.dram_tensor("xi", [S, MLOC * B], f16, kind="ExternalInput")
    # weights: per block 512 cols = [hr1 | hi2 | hi1 | hr2]
    w = nc.dram_tensor("w", [S, MLOC * 4 * S], f16, kind="ExternalInput")
    # y: 16 banks x 512 cols; bank k, partition 32*g+b (b<16), col 256*h+c
    # holds y[b, block k*8+h*4+g, c]
    y = nc.dram_tensor("y", [128, NBANK * 512], f16, kind="ExternalOutput")

    WGC = NB * 4 * S  # weight cols per DMA group (8192)

    with tile.TileContext(nc) as tc:
        with (
            tc.tile_pool(name="xp", bufs=1) as xpool,
            tc.tile_pool(name="wp", bufs=3) as wpool,
            tc.tile_pool(name="yp", bufs=2) as ypool,
            tc.tile_pool(name="ps", bufs=4, space=bass.MemorySpace.PSUM) as ps,
        ):
            xr_t = xpool.tile([S, MLOC * B], f16, name="xr_t")
            xi_t = xpool.tile([S, MLOC * B], f16, name="xi_t")
            # x loads ride the ACT ring so they overlap the first weight
            # group's DMA on the SP ring.
            nc.scalar.dma_start(xr_t[:], xr[:])
            nc.scalar.dma_start(xi_t[:], xi[:])

            yt = None
            for grp in range(NGRP):
                wt = wpool.tile([S, WGC], f16)
                nc.sync.dma_start(wt[:], w[:, grp * WGC:(grp + 1) * WGC])
                for b2 in range(NB // BPB):
                    bank = grp * (NB // BPB) + b2
                    q, r = divmod(bank, BANKS_PER_STORE)
                    if r == 0:
                        yt = ypool.tile([128, BANKS_PER_STORE * 512], f16)
                    pt = ps.tile([128, 512], f32)
                    for i in range(BPB):
                        il = b2 * BPB + i       # block within weight group
                        a = bank * BPB + i      # block within core shard
                        g, h = i % 4, i // 4
                        dst = pt[32 * g:32 * g + B, 256 * h:256 * (h + 1)]
                        w1 = wt[:, il * 512:il * 512 + 256]
                        w2 = wt[:, il * 512 + 256:(il + 1) * 512]
                        xs = slice(a * B, (a + 1) * B)
                        tp = (0, 32 * g)
                        nc.tensor.matmul(
                            dst, xr_t[:, xs], w1,
                            start=True, stop=False, tile_position=tp,
                        )
                        nc.tensor.matmul(
                            dst, xi_t[:, xs], w2,
                            start=False, stop=True, tile_position=tp,
                        )
                    nc.vector.tensor_copy(yt[:, r * 512:(r + 1) * 512], pt[:])
                    if r == BANKS_PER_STORE - 1:
                        c0 = q * BANKS_PER_STORE * 512
                        nc.scalar.dma_start(
                            y[:, c0:c0 + BANKS_PER_STORE * 512], yt[:]
                        )
    nc.compile()
    return nc


def kernel(x, hr1, hi1, hr2, hi2, perm_idx):
    from concourse.bass_utils import run_bass_kernel_spmd

    if "nc" not in _NC_CACHE:
        _NC_CACHE["nc"] = _build_nc()
    nc = _NC_CACHE["nc"]

    x = np.asarray(x, dtype=np.float32)
    perm_idx = np.asarray(perm_idx)
    # host-side permutation gather + regroup into M blocks of size S
    xp = x[:, :, perm_idx, :].reshape(B, 2, M, S).astype(np.float16)

    in_maps = []
    for c in range(NCORES):
        sl = slice(c * MLOC, (c + 1) * MLOC)
        # [B, MLOC, S] -> [S(j), MLOC, B] -> [S, MLOC*B]
        xre = np.ascontiguousarray(
            np.transpose(xp[:, 0, sl, :], (2, 1, 0))
        ).reshape(S, MLOC * B)
        xim = np.ascontiguousarray(
            np.transpose(xp[:, 1, sl, :], (2, 1, 0))
        ).reshape(S, MLOC * B)
        # per block 512 cols: [hr1 | hi2 | hi1 | hr2]
        wc = np.concatenate(
            [hr1[sl], hi2[sl], hi1[sl], hr2[sl]], axis=2
        ).astype(np.float16)                      # [MLOC, S, 512]
        wc = np.ascontiguousarray(np.transpose(wc, (1, 0, 2))).reshape(
            S, MLOC * 4 * S
        )
        in_maps.append({"xr": xre, "xi": xim, "w": wc})

    trace = bool(os.environ.get("KERNEL_TRACE"))
    kwargs = {}
    if trace:
        kwargs["tmpdir"] = os.environ.get("KERNEL_TRACE_DIR") or None
    res = run_bass_kernel_spmd(
        nc, in_maps, core_ids=list(range(NCORES)), trace=trace, **kwargs
    )
    if trace and res.exec_time_ns is not None:
        print(f"HW exec time: {res.exec_time_ns} ns")
        _NC_CACHE["exec_time_ns"] = res.exec_time_ns
        _NC_CACHE["profile"] = res

    out = np.empty((B, 2, M, S), dtype=np.float32)
    for c in range(NCORES):
        a0 = c * MLOC
        yd = res.results[c]["y"].reshape(4, 32, NBANK, 2, 256)[:, :B]
        # [g, b, bank, h, c] -> [b, bank, h, g, c]; block a = bank*8+h*4+g
        yc = np.transpose(yd, (1, 2, 3, 0, 4)).reshape(B, MLOC, 2 * S)
        yc = yc.astype(np.float32)
        out[:, 0, a0:a0 + MLOC, :] = yc[:, :, :S]
        out[:, 1, a0:a0 + MLOC, :] = yc[:, :, S:]
    return out.reshape(B, 2, N, R)


# revision 6
# speedup vs baseline: 2.5972x; 1.0402x over previous
"""Block-diagonal complex matmul kernel for trn2 (8 NeuronCores).

Reference computation:
  xp = take(x, perm_idx, axis=-2).reshape(B, 2, M, S)
  y_re = xp_re @ hr1 + xp_im @ hi1   (per block a of M)
  y_im = xp_re @ hi2 + xp_im @ hr2
  out  = stack([y_re, y_im], 1).reshape(B, 2, N, R)

Sharding: block dim M=1024 split across 8 cores (128 blocks each).
Permutation gather + all layout shuffles happen host-side in numpy.

Everything on-device is fp16 (correctness gate is 2e-2; fp16 gives ~1e-3).

Per-core device kernel, per block a:
  psum[16, 256] = x_re[:, a].T @ [hr1[a] | hi2[a]]   (start)
                + x_im[:, a].T @ [hi1[a] | hr2[a]]   (stop)
  -> cols 0:128 = y_re[a], cols 128:256 = y_im[a]

PSUM packing: 8 blocks per [128, 512] bank — block i at partition group
32*(i%4) (tensor-engine col tiling) and col half 256*(i//4).  One
128-partition DVE copy (fp32->fp16) per bank into an SBUF staging tile;
512KB output DMAs on the ACT ring (weights stream on the SP ring).
"""

import os
import numpy as np

B = 16
N = 4096
R = 32
M = 1024   # blocks
S = 128    # block size (contract dim)
NCORES = 8
MLOC = M // NCORES   # 128 blocks per core
NB = 8               # blocks per weight DMA group (1 MiB fp16)
NGRP = MLOC // NB    # 16 weight groups
BPB = 8              # blocks per PSUM bank
NBANK = MLOC // BPB  # 16 banks
BANKS_PER_STORE = 2  # y store granularity: [128, 1024] fp16 = 256 KiB

_NC_CACHE = {}


def _build_nc():
    import concourse.bacc as bacc
    import concourse.bass as bass
    import concourse.mybir as mybir
    from concourse import tile

    f16 = mybir.dt.float16
    f32 = mybir.dt.float32
    nc = bacc.Bacc(None, target_bir_lowering=False)

    # stationary x: col a*16+b holds x[b, block a, j=partition]
    xr = nc.dram_tensor("xr", [S, MLOC * B], f16, kind="ExternalInput")
    xi = nc.dram_tensor("xi", [S, MLOC * B], f16, kind="ExternalInput")
    # weights: per block 512 cols = [hr1 | hi2 | hi1 | hr2]
    w = nc.dram_tensor("w", [S, MLOC * 4 * S], f16, kind="ExternalInput")
    # y: 16 banks x 512 cols; bank k, partition 32*g+b (b<16), col 256*h+c
    # holds y[b, block k*8+h*4+g, c]
    y = nc.dram_tensor("y", [128, NBANK * 512], f16, kind="ExternalOutput")

    WGC = NB * 4 * S  # weight cols per DMA group (8192)

    with tile.TileContext(nc) as tc:
        with (
            tc.tile_pool(name="xp", bufs=1) as xpool,
            tc.tile_pool(name="wp", bufs=5) as wpool,
            tc.tile_pool(name="yp", bufs=2) as ypool,
            tc.tile_pool(name="ps", bufs=4, space=bass.MemorySpace.PSUM) as ps,
        ):
            xr_t = xpool.tile([S, MLOC * B], f16, name="xr_t")
            xi_t = xpool.tile([S, MLOC * B], f16, name="xi_t")
            # x loads go FIRST on the SP ring at full rate; the ACT ring's
            # 4 KiB packets lose the packet-granularity round-robin against
            # 16 KiB weight packets (~5x slowdown measured).
            nc.sync.dma_start(xr_t[:], xr[:])
            nc.sync.dma_start(xi_t[:], xi[:])

            yt = None
            for grp in range(NGRP):
                wt = wpool.tile([S, WGC], f16)
                nc.sync.dma_start(wt[:], w[:, grp * WGC:(grp + 1) * WGC])
                for b2 in range(NB // BPB):
                    bank = grp * (NB // BPB) + b2
                    q, r = divmod(bank, BANKS_PER_STORE)
                    if r == 0:
                        yt = ypool.tile([128, BANKS_PER_STORE * 512], f16)
                    pt = ps.tile([128, 512], f32)
                    for i in range(BPB):
                        il = b2 * BPB + i       # block within weight group
                        a = bank * BPB + i      # block within core shard
                        g, h = i % 4, i // 4
                        dst = pt[32 * g:32 * g + B, 256 * h:256 * (h + 1)]
                        w1 = wt[:, il * 512:il * 512 + 256]
                        w2 = wt[:, il * 512 + 256:(il + 1) * 512]
                        xs = slice(a * B, (a + 1) * B)
                        tp = (0, 32 * g)
                        nc.tensor.matmul(
                            dst, xr_t[:, xs], w1,
                            start=True, stop=False, tile_position=tp,
                        )
                        nc.tensor.matmul(
                            dst, xi_t[:, xs], w2,
                            start=False, stop=True, tile_position=tp,
                        )
                    nc.vector.tensor_copy(yt[:, r * 512:(r + 1) * 512], pt[:])
                    if r == BANKS_PER_STORE - 1:
                        c0 = q * BANKS_PER_STORE * 512
                        nc.scalar.dma_start(
                            y[:, c0:c0 + BANKS_PER_STORE * 512], yt[:]
                        )
    nc.compile()
    return nc


def kernel(x, hr1, hi1, hr2, hi2, perm_idx):
    from concourse.bass_utils import run_bass_kernel_spmd

    if "nc" not in _NC_CACHE:
        _NC_CACHE["nc"] = _build_nc()
    nc = _NC_CACHE["nc"]

    x = np.asarray(x, dtype=np.float32)
    perm_idx = np.asarray(perm_idx)
    # host-side permutation gather + regroup into M blocks of size S
    xp = x[:, :, perm_idx, :].reshape(B, 2, M, S).astype(np.float16)

    in_maps = []
    for c in range(NCORES):
        sl = slice(c * MLOC, (c + 1) * MLOC)
        # [B, MLOC, S] -> [S(j), MLOC, B] -> [S, MLOC*B]
        xre = np.ascontiguousarray(
            np.transpose(xp[:, 0, sl, :], (2, 1, 0))
        ).reshape(S, MLOC * B)
        xim = np.ascontiguousarray(
            np.transpose(xp[:, 1, sl, :], (2, 1, 0))
        ).reshape(S, MLOC * B)
        # per block 512 cols: [hr1 | hi2 | hi1 | hr2]
        wc = np.concatenate(
            [hr1[sl], hi2[sl], hi1[sl], hr2[sl]], axis=2
        ).astype(np.float16)                      # [MLOC, S, 512]
        wc = np.ascontiguousarray(np.transpose(wc, (1, 0, 2))).reshape(
            S, MLOC * 4 * S
        )
        in_maps.append({"xr": xre, "xi": xim, "w": wc})

    trace = bool(os.environ.get("KERNEL_TRACE"))
    kwargs = {}
    if trace:
        kwargs["tmpdir"] = os.environ.get("KERNEL_TRACE_DIR") or None
    res = run_bass_kernel_spmd(
        nc, in_maps, core_ids=list(range(NCORES)), trace=trace, **kwargs
    )
    if trace and res.exec_time_ns is not None:
        print(f"HW exec time: {res.exec_time_ns} ns")
        _NC_CACHE["exec_time_ns"] = res.exec_time_ns
        _NC_CACHE["profile"] = res

    out = np.empty((B, 2, M, S), dtype=np.float32)
    for c in range(NCORES):
        a0 = c * MLOC
        yd = res.results[c]["y"].reshape(4, 32, NBANK, 2, 256)[:, :B]
        # [g, b, bank, h, c] -> [b, bank, h, g, c]; block a = bank*8+h*4+g
        yc = np.transpose(yd, (1, 2, 3, 0, 4)).reshape(B, MLOC, 2 * S)
        yc = yc.astype(np.float32)
        out[:, 0, a0:a0 + MLOC, :] = yc[:, :, :S]
        out[:, 1, a0:a0 + MLOC, :] = yc[:, :, S:]
    return out.reshape(B, 2, N, R)
